# revision 22
# baseline (speedup 1.0000x reference)
"""BPR embedding-lookup kernel for 8 TRN2 NeuronCores.

Math (per batch element b):
    out[b] = dot(user_emb[users[b]], item_emb[items[b]])
           + sum_u social_weight[users[b], u] * dot(item_emb[items[b]], user_emb[u])

Per DISTINCT user j (the social row + pos term depend only on the user):
    W[:, j] = user_emb.T @ (social_weight[du[j], :] + onehot(du[j]))   # [64]
    out[b]  = dot(item_emb[items[b]], W[:, jmap[b]])
The onehot fold makes the PE accumulate V + bu in one pass.

Design (baseline transpose-gather 66us; v1 dense bf16 48.2us; this
version measures ~40-44us on HW, rel err 0.0139 vs the 2e-2 gate):
 - Global dedup: ~3364 distinct users of 4096 -> J~421 social rows per core
   (each core owns a contiguous range of distinct users and the batch
   elements that map to them). 17% fewer PE columns + DMA bytes.
 - social rows stored as fp8 e3m4 (1 byte): halves the dominant DMA stream
   vs bf16, and the PE ingests fp8e3 directly as the matmul rhs (no dequant
   stage; mixed bf16 lhsT x fp8e3 rhs verified exact on HW). e3m4's 4
   mantissa bits keep the quantization error acceptable (int8 dequant via
   DVE/ACT/Pool was tried and is too slow: 8-bit CASTs run ~2.4 cyc/elem and
   starve the PE).
 - W cols for distinct users are re-expanded to per-batch-element columns
   with a tiny selection matmul: out_blk[128b, 64d] = esel_jc_blk^T @ Wt_jc
   accumulated over j-chunks, where esel[j, b] = (jmap[b] == j). esel is a
   host-built 0/1 fp8e3 matrix; zero rows kill the padded/garbage W rows.
 - Final per-b dot: DVE mult + reduce against the indirect-gathered item
   embeddings (natural b-major layout). NOTE tensor_tensor_reduce (custom
   DVE op) crashes the exec unit in this flow - do not use it.

Host does layout only (shard, sort, quantize, pack); all FLOPs on device.
"""

import sys

if "/opt/trn_rl_repo" not in sys.path:
    sys.path.insert(0, "/opt/trn_rl_repo")

import numpy as np

NUM_USERS = 10000
NUM_ITEMS = 100000
D = 64
B = 4096
NCORES = 8
UK = 10112                # num_users padded to 79*128
KC = UK // 128            # 79 contraction chunks
# k-chunks per dense DMA slice: small first (PE starts after ~2 chunks land;
# tile deps are whole-tile), fat later (fewer+bigger DMA descriptors - the
# 1-byte stream is descriptor-rate-bound with small slices)
SLICES = [2, 2, 4, 8, 16, 16, 16, 15]
assert sum(SLICES) == KC

_PROGRAM_CACHE = {}
LAST_RESULTS = None


def _build_program(J: int, NB: int):
    """J: padded distinct-users-per-core (rhs cols). NB: 128-batch blocks."""
    import ml_dtypes  # noqa: F401

    from concourse import bacc, bass, mybir, tile

    f32 = mybir.dt.float32
    bf16 = mybir.dt.bfloat16
    fp8e3 = mybir.dt.float8e3
    i32 = mybir.dt.int32
    mult = mybir.AluOpType.mult
    add = mybir.AluOpType.add
    NCJ = (J + 127) // 128  # j-chunks for the esel selection matmul

    nc = bacc.Bacc(
        "TRN2",
        target_bir_lowering=False,
        debug=False,
        num_devices=NCORES,
    )
    # host-packed: swq[p, c*J + j] = fp8e3((social_weight[du[j]] + onehot)[c*128 + p])
    swq_d = nc.declare_dram_parameter("swq", [128, KC * J], fp8e3, isOutput=False)
    # host: uembk[p, c*D + d] = user_emb_padded[c*128 + p, d]  (bf16)
    uembk_d = nc.declare_dram_parameter("uembk", [128, KC * D], bf16, isOutput=False)
    # host: esel[j_in_chunk, ((jc*NB + blk)*128 + b)] = (jmap[blk*128+b] == jc*128+j)
    esel_d = nc.declare_dram_parameter("esel", [128, NCJ * NB * 128], fp8e3, isOutput=False)
    iemb_d = nc.declare_dram_parameter("iemb", [NUM_ITEMS, D], f32, isOutput=False)
    iidx_d = nc.declare_dram_parameter("iidx", [128, NB], i32, isOutput=False)
    out_d = nc.declare_dram_parameter("out", [128, NB], f32, isOutput=True)

    UEMBK_HEAD = 20  # k-chunks of uembk loaded first on the sync ring

    with tile.TileContext(nc) as tc:
        with (
            tc.tile_pool(name="const", bufs=1) as constp,
            tc.tile_pool(name="swq", bufs=1) as swqp,
            tc.tile_pool(name="small", bufs=4) as smallp,
            tc.tile_pool(name="psum", bufs=1, space="PSUM") as psump,
            tc.tile_pool(name="psum2", bufs=2, space="PSUM") as psum2p,
        ):
            # Best-measured sequencing (the first queue to hit the fabric
            # dominates it): iidx + uembk head + the whole social stream on
            # the sync ring; uembk tail + esel on the ACT ring, which wins
            # the fabric first for ~3us then hands over to the sync stream.
            iidx_t = constp.tile([128, NB], i32)
            nc.sync.dma_start(out=iidx_t[:], in_=iidx_d[:])
            uembk_h = constp.tile([128, UEMBK_HEAD, D], bf16)
            nc.sync.dma_start(
                out=uembk_h[:],
                in_=uembk_d[:, : UEMBK_HEAD * D].rearrange("p (c d) -> p c d", d=D),
            )
            uembk_r = constp.tile([128, KC - UEMBK_HEAD, D], bf16)
            esel_t = constp.tile([128, NCJ, NB, 128], fp8e3)

            # item-embedding gathers (SWDGE queue 0, tiny)
            bis = []
            for j in range(NB):
                bi = smallp.tile([128, D], f32, tag=f"bi{j}")
                nc.gpsimd.indirect_dma_start(
                    out=bi[:],
                    out_offset=None,
                    in_=iemb_d[:],
                    in_offset=bass.IndirectOffsetOnAxis(
                        ap=iidx_t[:, j : j + 1], axis=0
                    ),
                )
                bis.append(bi)

            swqs = []
            koff = 0
            for g, nch in enumerate(SLICES):
                swq = swqp.tile([128, nch, J], fp8e3, tag=f"swq{g}")
                nc.sync.dma_start(
                    out=swq[:],
                    in_=swq_d[
                        :, koff * J : (koff + nch) * J
                    ].rearrange("p (c j) -> p c j", j=J),
                )
                swqs.append(swq)
                koff += nch
                if g == 0:
                    gate = smallp.tile([128, 1], f32, tag="gate")
                    nc.scalar.copy(out=gate[:], in_=swq[:, 0, :1])
                    nc.scalar.dma_start(
                        out=uembk_r[:],
                        in_=uembk_d[:, UEMBK_HEAD * D :].rearrange(
                            "p (c d) -> p c d", d=D
                        ),
                    )
                    nc.scalar.dma_start(
                        out=esel_t[:],
                        in_=esel_d[:].rearrange(
                            "p (c n b) -> p c n b", n=NB, b=128
                        ),
                    )

            ident = constp.tile([D, D], f32)
            from concourse.masks import make_identity

            make_identity(nc, ident[:])

            # W^T[d, j] accumulated over all 79 k-chunks; the PE ingests the
            # fp8e3 social slices directly (mixed bf16 lhsT x fp8e3 rhs, HW
            # verified exact). Two interleaved PSUM accumulation chains hide
            # LDWEIGHTS under the other chain's MATMUL.
            vt_ps0 = psump.tile([D, J], f32, tag="vt0")
            vt_ps1 = psump.tile([D, J], f32, tag="vt1")
            chains = [vt_ps0, vt_ps1]

            slice_of = []
            for g, nch in enumerate(SLICES):
                slice_of += [(g, c) for c in range(nch)]
            for kchunk in range(KC):
                g, c = slice_of[kchunk]
                if kchunk < UEMBK_HEAD:
                    lhsT = uembk_h[:, kchunk, :]
                else:
                    lhsT = uembk_r[:, kchunk - UEMBK_HEAD, :]
                nc.tensor.matmul(
                    out=chains[kchunk % 2][:],
                    lhsT=lhsT,
                    rhs=swqs[g][:, c, :],
                    start=(kchunk < 2),
                    stop=(kchunk >= KC - 2),
                )

            # stage W^T = chain0 + chain1 into SBUF (DVE can read one PSUM
            # operand per op), then transpose to [128j, 64d]; per-j-chunk so
            # the DVE copy/add pipelines under the PE transposes
            vt_a = constp.tile([D, J], f32)
            vt_sb = constp.tile([D, J], f32)
            wt_sb = constp.tile([128, NCJ, D], bf16)
            for jc in range(NCJ):
                jn = min(128, J - jc * 128)
                sl = slice(jc * 128, jc * 128 + jn)
                nc.vector.tensor_copy(vt_a[:, sl], vt_ps0[:, sl])
                nc.vector.tensor_tensor(
                    out=vt_sb[:, sl], in0=vt_a[:, sl], in1=vt_ps1[:, sl], op=add
                )
                t_ps = psum2p.tile([128, D], f32, tag="tps")
                nc.tensor.transpose(
                    out=t_ps[:jn, :],
                    in_=vt_sb[:, sl],
                    identity=ident[:],
                )
                nc.vector.tensor_copy(wt_sb[:jn, jc, :], t_ps[:jn, :])

            # re-expand to per-batch-element columns: out_blk[128b, 64d] =
            # sum_jc esel_jc_blk^T @ wt_jc   (esel zero rows kill padded W rows)
            out_stage = constp.tile([128, NB], f32)
            wsel0 = psum2p.tile([128, D], f32, tag="wsel0")
            wsel1 = psum2p.tile([128, D], f32, tag="wsel1")
            wsel_chains = [wsel0, wsel1]
            for blk in range(NB):
                wsel = wsel_chains[blk % 2]
                for jc in range(NCJ):
                    jn = min(128, J - jc * 128)
                    nc.tensor.matmul(
                        out=wsel[:],
                        lhsT=esel_t[:jn, jc, blk, :],
                        rhs=wt_sb[:jn, jc, :],
                        start=(jc == 0),
                        stop=(jc == NCJ - 1),
                    )
                prod = smallp.tile([128, D], f32, tag="prod")
                nc.vector.tensor_tensor(
                    out=prod[:], in0=bis[blk][:], in1=wsel[:], op=mult
                )
                nc.vector.tensor_reduce(
                    out=out_stage[:, blk : blk + 1],
                    in_=prod[:],
                    axis=mybir.AxisListType.X,
                    op=add,
                )
                nc.sync.dma_start(
                    out=out_d[:, blk : blk + 1], in_=out_stage[:, blk : blk + 1]
                )

    nc.finalize()
    return nc


def kernel(user_emb, item_emb, social_weight, users, items):
    global LAST_RESULTS
    import os

    import ml_dtypes

    from concourse.bass_utils import run_bass_kernel_spmd

    bf = ml_dtypes.bfloat16
    e3m4 = ml_dtypes.float8_e3m4
    user_emb = np.ascontiguousarray(np.asarray(user_emb, dtype=np.float32))
    item_emb = np.ascontiguousarray(np.asarray(item_emb, dtype=np.float32))
    social_weight = np.ascontiguousarray(np.asarray(social_weight, dtype=np.float32))
    users = np.asarray(users).astype(np.int64)
    items = np.asarray(items).astype(np.int64)

    # global dedup; core m owns a contiguous range of distinct users and the
    # batch elements mapping to them
    du, jmap_g = np.unique(users, return_inverse=True)  # du sorted, len ND
    ND = len(du)
    J = (ND + NCORES - 1) // NCORES          # distinct users per core
    J = (J + 7) // 8 * 8                     # pad to multiple of 8
    bounds = [min(m * J, ND) for m in range(NCORES + 1)]

    # batch elements per core (order: sorted by jmap); inverse permutation
    order = np.argsort(jmap_g, kind="stable")
    jmap_s = jmap_g[order]
    items_s = items[order].astype(np.int32)
    core_of = np.minimum(jmap_s // J, NCORES - 1)
    counts = np.bincount(core_of, minlength=NCORES)
    NBmax = int(np.max(counts))
    NB = (NBmax + 127) // 128

    key = (J, NB)
    if key not in _PROGRAM_CACHE:
        _PROGRAM_CACHE[key] = _build_program(J, NB)
    nc = _PROGRAM_CACHE[key]

    uembk_pad = np.zeros((UK, D), bf)
    uembk_pad[:NUM_USERS] = user_emb.astype(bf)
    uembk = np.ascontiguousarray(
        uembk_pad.reshape(KC, 128, D).transpose(1, 0, 2).reshape(128, KC * D)
    )
    NCJ = (J + 127) // 128

    in_maps = []
    starts = np.concatenate([[0], np.cumsum(counts)])
    for m in range(NCORES):
        lo, hi = bounds[m], bounds[m + 1]
        nj = hi - lo
        du_m = du[lo:hi]
        # quantized social rows for this core's distinct users (+onehot fold)
        rows = social_weight[du_m]  # [nj, NUM_USERS] f32 copy
        rows[np.arange(nj), du_m] += np.float32(1.0)
        arr = np.zeros((J, UK), e3m4)
        arr[:nj, :NUM_USERS] = rows.astype(e3m4)
        swq = np.ascontiguousarray(
            arr.reshape(J, KC, 128).transpose(2, 1, 0).reshape(128, KC * J)
        )

        bm = counts[m]
        jl = jmap_s[starts[m] : starts[m + 1]] - lo  # local j of each b
        it_m = np.zeros(NB * 128, np.int32)
        it_m[:bm] = items_s[starts[m] : starts[m + 1]]
        esel = np.zeros((128, NCJ, NB, 128), e3m4)
        b_idx = np.arange(bm)
        esel[jl % 128, jl // 128, b_idx // 128, b_idx % 128] = np.float32(1.0)

        in_maps.append(
            {
                "swq": swq,
                "uembk": uembk,
                "esel": np.ascontiguousarray(esel.reshape(128, NCJ * NB * 128)),
                "iemb": item_emb,
                "iidx": np.ascontiguousarray(it_m.reshape(NB, 128).T),
            }
        )

    trace = bool(os.environ.get("CC_KERNEL_TRACE"))
    tmpdir = os.environ.get("CC_TRACE_DIR") or None
    res = run_bass_kernel_spmd(
        nc, in_maps, list(range(NCORES)), trace=trace, tmpdir=tmpdir
    )
    LAST_RESULTS = res

    out_sorted = np.empty(B, np.float32)
    for m in range(NCORES):
        o = np.asarray(res.results[m]["out"])  # [128, NB]
        out_sorted[starts[m] : starts[m + 1]] = o.T.reshape(-1)[: counts[m]]

    final = np.empty(B, np.float32)
    final[order] = out_sorted
    return final


# revision 23
# speedup vs baseline: 1.1076x; 1.1076x over previous
"""BPR embedding-lookup kernel for 8 TRN2 NeuronCores.

Math (per batch element b):
    out[b] = dot(user_emb[users[b]], item_emb[items[b]])
           + sum_u social_weight[users[b], u] * dot(item_emb[items[b]], user_emb[u])

Per DISTINCT user j (the social row + pos term depend only on the user):
    W[:, j] = user_emb.T @ (social_weight[du[j], :] + onehot(du[j]))   # [64]
    out[b]  = dot(item_emb[items[b]], W[:, jmap[b]])
The onehot fold makes the PE accumulate V + bu in one pass.

Design (baseline transpose-gather 66us; v1 dense bf16 48.2us; this
version measures ~40-44us on HW, rel err 0.0139 vs the 2e-2 gate):
 - Global dedup: ~3364 distinct users of 4096 -> J~421 social rows per core
   (each core owns a contiguous range of distinct users and the batch
   elements that map to them). 17% fewer PE columns + DMA bytes.
 - social rows stored as fp8 e3m4 (1 byte): halves the dominant DMA stream
   vs bf16, and the PE ingests fp8e3 directly as the matmul rhs (no dequant
   stage; mixed bf16 lhsT x fp8e3 rhs verified exact on HW). e3m4's 4
   mantissa bits keep the quantization error acceptable (int8 dequant via
   DVE/ACT/Pool was tried and is too slow: 8-bit CASTs run ~2.4 cyc/elem and
   starve the PE).
 - W cols for distinct users are re-expanded to per-batch-element columns
   with a tiny selection matmul: out_blk[128b, 64d] = esel_jc_blk^T @ Wt_jc
   accumulated over j-chunks, where esel[j, b] = (jmap[b] == j). esel is a
   host-built 0/1 fp8e3 matrix; zero rows kill the padded/garbage W rows.
 - Final per-b dot: DVE mult + reduce against the indirect-gathered item
   embeddings (natural b-major layout). NOTE tensor_tensor_reduce (custom
   DVE op) crashes the exec unit in this flow - do not use it.

Host does layout only (shard, sort, quantize, pack); all FLOPs on device.
"""

import sys

if "/opt/trn_rl_repo" not in sys.path:
    sys.path.insert(0, "/opt/trn_rl_repo")

import numpy as np

NUM_USERS = 10000
NUM_ITEMS = 100000
D = 64
B = 4096
NCORES = 8
UK = 10112                # num_users padded to 79*128
KC = UK // 128            # 79 contraction chunks
# k-chunks per dense DMA slice: small first (PE starts after ~2 chunks land;
# tile deps are whole-tile), fat later (fewer+bigger DMA descriptors - the
# 1-byte stream is descriptor-rate-bound with small slices)
SLICES = [2, 2, 4, 8, 16, 16, 16, 15]
assert sum(SLICES) == KC

_PROGRAM_CACHE = {}
LAST_RESULTS = None


def _build_program(J: int, NB: int):
    """J: padded distinct-users-per-core (rhs cols). NB: 128-batch blocks."""
    import ml_dtypes  # noqa: F401

    from concourse import bacc, bass, mybir, tile

    f32 = mybir.dt.float32
    bf16 = mybir.dt.bfloat16
    fp8e3 = mybir.dt.float8e3
    i32 = mybir.dt.int32
    mult = mybir.AluOpType.mult
    add = mybir.AluOpType.add
    NCJ = (J + 127) // 128  # j-chunks for the esel selection matmul

    nc = bacc.Bacc(
        "TRN2",
        target_bir_lowering=False,
        debug=False,
        num_devices=NCORES,
    )
    # host-packed: swq[p, c*J + j] = fp8e3((social_weight[du[j]] + onehot)[c*128 + p])
    swq_d = nc.declare_dram_parameter("swq", [128, KC * J], fp8e3, isOutput=False)
    # host: uembk[p, c*D + d] = user_emb_padded[c*128 + p, d]  (bf16)
    uembk_d = nc.declare_dram_parameter("uembk", [128, KC * D], bf16, isOutput=False)
    # host: esel[j_in_chunk, ((jc*NB + blk)*128 + b)] = (jmap[blk*128+b] == jc*128+j)
    esel_d = nc.declare_dram_parameter("esel", [128, NCJ * NB * 128], fp8e3, isOutput=False)
    iemb_d = nc.declare_dram_parameter("iemb", [NUM_ITEMS, D], f32, isOutput=False)
    iidx_d = nc.declare_dram_parameter("iidx", [128, NB], i32, isOutput=False)
    out_d = nc.declare_dram_parameter("out", [128, NB], f32, isOutput=True)

    UEMBK_HEAD = 20  # k-chunks of uembk loaded first on the sync ring

    with tile.TileContext(nc) as tc:
        with (
            tc.tile_pool(name="const", bufs=1) as constp,
            tc.tile_pool(name="swq", bufs=1) as swqp,
            tc.tile_pool(name="small", bufs=4) as smallp,
            tc.tile_pool(name="psum", bufs=1, space="PSUM") as psump,
            tc.tile_pool(name="psum2", bufs=2, space="PSUM") as psum2p,
        ):
            # Best-measured sequencing (the first queue to hit the fabric
            # dominates it): iidx + uembk head + the whole social stream on
            # the sync ring; uembk tail + esel on the ACT ring, which wins
            # the fabric first for ~3us then hands over to the sync stream.
            iidx_t = constp.tile([128, NB], i32)
            nc.sync.dma_start(out=iidx_t[:], in_=iidx_d[:])
            uembk_h = constp.tile([128, UEMBK_HEAD, D], bf16)
            nc.sync.dma_start(
                out=uembk_h[:],
                in_=uembk_d[:, : UEMBK_HEAD * D].rearrange("p (c d) -> p c d", d=D),
            )
            uembk_r = constp.tile([128, KC - UEMBK_HEAD, D], bf16)
            esel_t = constp.tile([128, NCJ, NB, 128], fp8e3)

            # item-embedding gathers (SWDGE queue 0, tiny)
            bis = []
            for j in range(NB):
                bi = smallp.tile([128, D], f32, tag=f"bi{j}")
                nc.gpsimd.indirect_dma_start(
                    out=bi[:],
                    out_offset=None,
                    in_=iemb_d[:],
                    in_offset=bass.IndirectOffsetOnAxis(
                        ap=iidx_t[:, j : j + 1], axis=0
                    ),
                )
                bis.append(bi)

            swqs = []
            koff = 0
            for g, nch in enumerate(SLICES):
                swq = swqp.tile([128, nch, J], fp8e3, tag=f"swq{g}")
                nc.sync.dma_start(
                    out=swq[:],
                    in_=swq_d[
                        :, koff * J : (koff + nch) * J
                    ].rearrange("p (c j) -> p c j", j=J),
                )
                swqs.append(swq)
                koff += nch
                if g == 0:
                    gate = smallp.tile([128, 1], f32, tag="gate")
                    nc.scalar.copy(out=gate[:], in_=swq[:, 0, :1])
                    nc.scalar.dma_start(
                        out=uembk_r[:],
                        in_=uembk_d[:, UEMBK_HEAD * D :].rearrange(
                            "p (c d) -> p c d", d=D
                        ),
                    )
                    nc.scalar.dma_start(
                        out=esel_t[:],
                        in_=esel_d[:].rearrange(
                            "p (c n b) -> p c n b", n=NB, b=128
                        ),
                    )

            ident = constp.tile([D, D], f32)
            from concourse.masks import make_identity

            make_identity(nc, ident[:])

            # W^T[d, j] accumulated over all 79 k-chunks; the PE ingests the
            # fp8e3 social slices directly (mixed bf16 lhsT x fp8e3 rhs, HW
            # verified exact). Two interleaved PSUM accumulation chains hide
            # LDWEIGHTS under the other chain's MATMUL.
            vt_ps0 = psump.tile([D, J], f32, tag="vt0")
            vt_ps1 = psump.tile([D, J], f32, tag="vt1")
            chains = [vt_ps0, vt_ps1]

            slice_of = []
            for g, nch in enumerate(SLICES):
                slice_of += [(g, c) for c in range(nch)]
            for kchunk in range(KC):
                g, c = slice_of[kchunk]
                if kchunk < UEMBK_HEAD:
                    lhsT = uembk_h[:, kchunk, :]
                else:
                    lhsT = uembk_r[:, kchunk - UEMBK_HEAD, :]
                nc.tensor.matmul(
                    out=chains[kchunk % 2][:],
                    lhsT=lhsT,
                    rhs=swqs[g][:, c, :],
                    start=(kchunk < 2),
                    stop=(kchunk >= KC - 2),
                )

            # stage W^T = chain0 + chain1 into SBUF (DVE can read one PSUM
            # operand per op), then transpose j-chunks to [128j, 64d]
            vt_a = constp.tile([D, J], f32)
            nc.vector.tensor_copy(vt_a[:], vt_ps0[:])
            vt_sb = constp.tile([D, J], f32)
            nc.vector.tensor_tensor(out=vt_sb[:], in0=vt_a[:], in1=vt_ps1[:], op=add)
            wt_sb = constp.tile([128, NCJ, D], bf16)
            for jc in range(NCJ):
                jn = min(128, J - jc * 128)
                t_ps = psum2p.tile([128, D], f32, tag="tps")
                nc.tensor.transpose(
                    out=t_ps[:jn, :],
                    in_=vt_sb[:, jc * 128 : jc * 128 + jn],
                    identity=ident[:],
                )
                nc.vector.tensor_copy(wt_sb[:jn, jc, :], t_ps[:jn, :])

            # re-expand to per-batch-element columns: out_blk[128b, 64d] =
            # sum_jc esel_jc_blk^T @ wt_jc   (esel zero rows kill padded W rows)
            out_stage = constp.tile([128, NB], f32)
            wsel0 = psum2p.tile([128, D], f32, tag="wsel0")
            wsel1 = psum2p.tile([128, D], f32, tag="wsel1")
            wsel_chains = [wsel0, wsel1]
            for blk in range(NB):
                wsel = wsel_chains[blk % 2]
                for jc in range(NCJ):
                    jn = min(128, J - jc * 128)
                    nc.tensor.matmul(
                        out=wsel[:],
                        lhsT=esel_t[:jn, jc, blk, :],
                        rhs=wt_sb[:jn, jc, :],
                        start=(jc == 0),
                        stop=(jc == NCJ - 1),
                    )
                prod = smallp.tile([128, D], f32, tag="prod")
                nc.vector.tensor_tensor(
                    out=prod[:], in0=bis[blk][:], in1=wsel[:], op=mult
                )
                nc.vector.tensor_reduce(
                    out=out_stage[:, blk : blk + 1],
                    in_=prod[:],
                    axis=mybir.AxisListType.X,
                    op=add,
                )
            nc.sync.dma_start(out=out_d[:], in_=out_stage[:])

    nc.finalize()
    return nc


def kernel(user_emb, item_emb, social_weight, users, items):
    global LAST_RESULTS
    import os

    import ml_dtypes

    from concourse.bass_utils import run_bass_kernel_spmd

    bf = ml_dtypes.bfloat16
    e3m4 = ml_dtypes.float8_e3m4
    user_emb = np.ascontiguousarray(np.asarray(user_emb, dtype=np.float32))
    item_emb = np.ascontiguousarray(np.asarray(item_emb, dtype=np.float32))
    social_weight = np.ascontiguousarray(np.asarray(social_weight, dtype=np.float32))
    users = np.asarray(users).astype(np.int64)
    items = np.asarray(items).astype(np.int64)

    # global dedup; core m owns a contiguous range of distinct users and the
    # batch elements mapping to them
    du, jmap_g = np.unique(users, return_inverse=True)  # du sorted, len ND
    ND = len(du)
    J = (ND + NCORES - 1) // NCORES          # distinct users per core
    J = (J + 7) // 8 * 8                     # pad to multiple of 8
    bounds = [min(m * J, ND) for m in range(NCORES + 1)]

    # batch elements per core (order: sorted by jmap); inverse permutation
    order = np.argsort(jmap_g, kind="stable")
    jmap_s = jmap_g[order]
    items_s = items[order].astype(np.int32)
    core_of = np.minimum(jmap_s // J, NCORES - 1)
    counts = np.bincount(core_of, minlength=NCORES)
    NBmax = int(np.max(counts))
    NB = (NBmax + 127) // 128

    key = (J, NB)
    if key not in _PROGRAM_CACHE:
        _PROGRAM_CACHE[key] = _build_program(J, NB)
    nc = _PROGRAM_CACHE[key]

    uembk_pad = np.zeros((UK, D), bf)
    uembk_pad[:NUM_USERS] = user_emb.astype(bf)
    uembk = np.ascontiguousarray(
        uembk_pad.reshape(KC, 128, D).transpose(1, 0, 2).reshape(128, KC * D)
    )
    NCJ = (J + 127) // 128

    in_maps = []
    starts = np.concatenate([[0], np.cumsum(counts)])
    for m in range(NCORES):
        lo, hi = bounds[m], bounds[m + 1]
        nj = hi - lo
        du_m = du[lo:hi]
        # quantized social rows for this core's distinct users (+onehot fold)
        rows = social_weight[du_m]  # [nj, NUM_USERS] f32 copy
        rows[np.arange(nj), du_m] += np.float32(1.0)
        arr = np.zeros((J, UK), e3m4)
        arr[:nj, :NUM_USERS] = rows.astype(e3m4)
        swq = np.ascontiguousarray(
            arr.reshape(J, KC, 128).transpose(2, 1, 0).reshape(128, KC * J)
        )

        bm = counts[m]
        jl = jmap_s[starts[m] : starts[m + 1]] - lo  # local j of each b
        it_m = np.zeros(NB * 128, np.int32)
        it_m[:bm] = items_s[starts[m] : starts[m + 1]]
        esel = np.zeros((128, NCJ, NB, 128), e3m4)
        b_idx = np.arange(bm)
        esel[jl % 128, jl // 128, b_idx // 128, b_idx % 128] = np.float32(1.0)

        in_maps.append(
            {
                "swq": swq,
                "uembk": uembk,
                "esel": np.ascontiguousarray(esel.reshape(128, NCJ * NB * 128)),
                "iemb": item_emb,
                "iidx": np.ascontiguousarray(it_m.reshape(NB, 128).T),
            }
        )

    trace = bool(os.environ.get("CC_KERNEL_TRACE"))
    tmpdir = os.environ.get("CC_TRACE_DIR") or None
    res = run_bass_kernel_spmd(
        nc, in_maps, list(range(NCORES)), trace=trace, tmpdir=tmpdir
    )
    LAST_RESULTS = res

    out_sorted = np.empty(B, np.float32)
    for m in range(NCORES):
        o = np.asarray(res.results[m]["out"])  # [128, NB]
        out_sorted[starts[m] : starts[m + 1]] = o.T.reshape(-1)[: counts[m]]

    final = np.empty(B, np.float32)
    final[order] = out_sorted
    return final
